# revision 53
# baseline (speedup 1.0000x reference)
"""Trainium2 Bass kernel for the CRW intrinsic-reward loss.

Computation (see reference): two branches (state / next_state) through
BatchNorm(full batch) -> clip -> 3-layer MLP -> s, t [B, 512]; then
loss = -sum_{b,i} log( sum_j A^2 ) with A = softmax_j(s_i * t_j).

Key identity used on device (row-max cancels exactly):
    log(sum_j A^2) = log(sum_j e^{2 s_i t_j}) - 2 log(sum_j e^{s_i t_j})
so  loss = sum_{b,i} [ 2 ln(S1) - ln(S2) ],  S1 = sum_j e^{s_i t_j},
    S2 = sum_j (e^{s_i t_j})^2.

Sharding: data-parallel over batch, B=512 -> 64 samples/core on 8 cores.
Full (transposed) inputs are replicated so each core computes the full-batch
BatchNorm statistics locally; MLP weights replicated (W1 bf16, W2/W3 fp8-e4m3
pre-scaled by 256 with the descale folded into the PSUM->SBUF evictions);
each core emits a [128,1] vector of partial loss sums, summed on the host.
"""

import numpy as np
import ml_dtypes

import concourse.bacc as bacc
import concourse.tile as tile
import concourse.mybir as mybir
from concourse.bass_utils import run_bass_kernel_spmd

F32 = mybir.dt.float32
BF16 = mybir.dt.bfloat16
F8 = mybir.dt.float8e4
WSCALE = 256.0
AF = mybir.ActivationFunctionType
OP = mybir.AluOpType

EPS = 1e-5
CLIP = 5.0
B, OBS, HID, REP = 512, 64, 1024, 512
NCORES = 8
BS = B // NCORES  # 64 samples per core


def build_program():
    nc = bacc.Bacc("TRN2", target_bir_lowering=False, debug=False)

    xyT = nc.dram_tensor("xyT", [OBS, 2 * B], BF16, kind="ExternalInput").ap()
    xycT = nc.dram_tensor("xycT", [OBS, 2 * BS], BF16, kind="ExternalInput").ap()
    w1 = nc.dram_tensor("w1", [OBS, HID], BF16, kind="ExternalInput").ap()
    w2 = nc.dram_tensor("w2", [HID, HID], F8, kind="ExternalInput").ap()
    w3 = nc.dram_tensor("w3", [HID, REP], F8, kind="ExternalInput").ap()
    bcat = nc.dram_tensor("bcat", [2 * HID + REP], F32, kind="ExternalInput").ap()
    v_out = nc.dram_tensor("v", [128, 1], F32, kind="ExternalOutput").ap()

    with tile.TileContext(nc) as tc:
        with (
            tc.tile_pool(name="const", bufs=1) as const,
            tc.tile_pool(name="w", bufs=1) as wpool,
            tc.tile_pool(name="xin", bufs=1) as xpool,
            tc.tile_pool(name="norm", bufs=2) as npool,
            tc.tile_pool(name="st", bufs=1) as spool,
            tc.tile_pool(name="sums", bufs=1) as sums,
        ):
            # ---- input DMAs; ordering matters: the queue issues serially
            # (~650ns each) so front-load what the critical chain needs ----
            NB = 2 * HID + REP
            xyT_sb = xpool.tile([OBS, 2, B], BF16, tag="xyT")
            xyc_sb = xpool.tile([OBS, 2 * BS], BF16, tag="xyc")
            bf_sb = const.tile([1, NB], F32, tag="bf")
            w1_sb = wpool.tile([OBS, HID], BF16, tag="w1")
            w2_sb = wpool.tile([128, 8, HID], F8, tag="w2")
            w3_sb = wpool.tile([128, 8, REP], F8, tag="w3")
            w2r = w2.rearrange("(t p) n -> p t n", p=128)
            xyTr = xyT.rearrange("f (h b) -> f h b", h=2)

            w3r = w3.rearrange("(t p) n -> p t n", p=128)
            nc.sync.dma_start(out=bf_sb, in_=bcat.rearrange("(o n) -> o n", o=1))
            nc.sync.dma_start(out=xyT_sb, in_=xyTr)
            nc.sync.dma_start(out=xyc_sb, in_=xycT)
            nc.sync.dma_start(out=w1_sb, in_=w1)
            nc.sync.dma_start(out=w2_sb[:, 0:4, :], in_=w2r[:, 0:4, :])
            nc.sync.dma_start(out=w2_sb[:, 4:8, :], in_=w2r[:, 4:8, :])
            nc.sync.dma_start(out=w3_sb, in_=w3r)

            ball_sb = const.tile([1, NB], BF16, tag="ball")
            nc.scalar.copy(ball_sb, bf_sb)  # ACT is idle here; DVE is not
            b1_sb = ball_sb[0:1, 0:HID]
            b2_sb = ball_sb[0:1, HID:2 * HID]
            b3_sb = ball_sb[0:1, 2 * HID:NB]
            ones_sb = const.tile([1, 2 * BS], BF16, tag="ones")
            nc.vector.memset(ones_sb, 1.0)
            eps_sb = const.tile([OBS, 1], F32, tag="eps")
            nc.vector.memset(eps_sb, EPS)
            # dummy sqrt: hoists the sqrt ACT-table load off the critical path
            dummy = const.tile([1, 1], F32, tag="dummy")
            nc.vector.memset(dummy, 1.0)
            nc.scalar.activation(out=dummy, in_=dummy, func=AF.Sqrt)
            # PE warm-up burst during the DMA window: ~3.5us of continuous PE
            # work un-throttles HAM before the MLP needs full speed
            warm_src = const.tile([1, REP], BF16, tag="warm_src")
            nc.vector.memset(warm_src, 0.0)
            with tc.tile_pool(name="ps_warm", bufs=1, space="PSUM") as ps_warm:
                warm_ps = ps_warm.tile([1, REP], F32, tag="warm")
                for _ in range(10):
                    nc.tensor.matmul(
                        warm_ps, warm_src[0:1, 0:1], warm_src,
                        start=True, stop=True,
                    )

            # ---- BatchNorm (full-batch stats) + clip; both branches share
            # one concatenated activation tile zc_cat [64, 128] (s | t) ----
            M2 = 2 * BS  # 128 samples: both branches concatenated
            zc_cat = npool.tile([OBS, M2], BF16, tag="zc_cat")

            mv2 = npool.tile([OBS, 2, 2], F32, tag="bnmv")
            for half in range(2):
                st = npool.tile([OBS, 6], F32, tag="bnst")
                nc.vector.bn_stats(out=st, in_=xyT_sb[:, half, :])
                nc.vector.bn_aggr(out=mv2[:, half, :], in_=st)
            sig2 = npool.tile([OBS, 2], F32, tag="sig")
            nc.scalar.activation(
                out=sig2, in_=mv2[:, :, 1], func=AF.Sqrt, bias=eps_sb)
            rstd2 = npool.tile([OBS, 2], F32, tag="rstd")
            rscr = npool.tile([OBS, 2], F32, tag="rscr")
            nc.vector.reciprocal_approx_accurate(out=rstd2, in_=sig2, scratch=rscr)
            for half in range(2):
                z = npool.tile([OBS, BS], F32, tag="z")
                nc.vector.tensor_scalar(
                    out=z, in0=xyc_sb[:, half * BS:(half + 1) * BS],
                    scalar1=mv2[:, half, 0:1], scalar2=rstd2[:, half:half + 1],
                    op0=OP.subtract, op1=OP.mult,
                )
                nc.vector.tensor_scalar(
                    out=zc_cat[:, half * BS:(half + 1) * BS], in0=z,
                    scalar1=CLIP, scalar2=-CLIP, op0=OP.min, op1=OP.max,
                )
            sig1 = sig2
            # dummy exp AFTER the last sqrt (data dep pins the order): swaps
            # the ACT table to natural_log_exp early, while the MLP
            # (relu-only, present in every set) runs
            nc.scalar.activation(out=dummy, in_=sig1[0:1, 0:1], func=AF.Exp)

            # ---- 3-layer MLP, both branches in one pass ----
            # flat single-partition copies: matmul operands need base partition 0
            s_flat = spool.tile([1, BS * REP], BF16, tag="sflat")
            t_flat = spool.tile([1, BS * REP], BF16, tag="tflat")

            with (
                tc.tile_pool(name="mlp", bufs=2) as mlp,
                tc.tile_pool(name="ps_mlp", bufs=4, space="PSUM") as ps_mlp,
                tc.tile_pool(name="ps_s", bufs=1, space="PSUM") as ps_s,
            ):
                h1 = mlp.tile([128, 8 * M2], BF16, tag="h1")
                for n in range(8):
                    ps = ps_mlp.tile([128, M2], F32, tag="ps")
                    nc.tensor.matmul(
                        ps, w1_sb[:, 128 * n:128 * (n + 1)], zc_cat,
                        start=True, stop=False,
                    )
                    nc.tensor.matmul(
                        ps, b1_sb[0:1, 128 * n:128 * (n + 1)], ones_sb,
                        start=False, stop=True,
                    )
                    if n % 2 == 0:
                        nc.vector.tensor_scalar(
                            out=h1[:, M2 * n:M2 * (n + 1)], in0=ps,
                            scalar1=0.0, scalar2=None, op0=OP.max,
                        )
                    else:
                        nc.scalar.activation(
                            out=h1[:, M2 * n:M2 * (n + 1)], in_=ps, func=AF.Relu,
                        )
                h2 = mlp.tile([128, 8 * M2], BF16, tag="h2")
                for n in range(8):
                    ps = ps_mlp.tile([128, M2], F32, tag="ps")
                    for kt in range(8):
                        nc.tensor.matmul(
                            ps, w2_sb[:, kt, 128 * n:128 * (n + 1)],
                            h1[:, M2 * kt:M2 * (kt + 1)],
                            start=(kt == 0), stop=False,
                        )
                    nc.tensor.matmul(
                        ps, b2_sb[0:1, 128 * n:128 * (n + 1)], ones_sb,
                        start=False, stop=True,
                    )
                    nc.vector.tensor_scalar(
                        out=h2[:, M2 * n:M2 * (n + 1)], in0=ps,
                        scalar1=1.0 / WSCALE, scalar2=0.0,
                        op0=OP.mult, op1=OP.max,
                    )
                ps3 = ps_s.tile([M2, REP], F32, tag="ps3")
                for kt in range(8):
                    nc.tensor.matmul(
                        ps3, h2[:, M2 * kt:M2 * (kt + 1)], w3_sb[:, kt, :],
                        start=(kt == 0), stop=False,
                    )
                nc.tensor.matmul(ps3, ones_sb, b3_sb, start=False, stop=True)
                # split copies: ACT takes the s half, idle DVE the t half --
                # two separate tiles put sample 0's s and t rows both at base
                # partition 0 so its outer products can skip the flat DMAs
                s_bf2 = spool.tile([BS, REP], BF16, tag="s2")
                t_bf2 = spool.tile([BS, REP], BF16, tag="t2")
                nc.scalar.mul(s_bf2, ps3[0:BS, :], 1.0 / WSCALE)
                nc.scalar.mul(t_bf2, ps3[BS:M2, :], 1.0 / WSCALE)
                nc.sync.dma_start(out=s_flat, in_=s_bf2)
                nc.sync.dma_start(out=t_flat, in_=t_bf2)
                # keep PE warm across the L3 -> flat-DMA handoff gap
                warm_ps2 = ps_mlp.tile([1, REP], F32, tag="ps")
                for _ in range(6):
                    nc.tensor.matmul(
                        warm_ps2, warm_src[0:1, 0:1], warm_src,
                        start=True, stop=True,
                    )

            # ---- stage 2: per-sample rank-1 scores, exp, row sums ----
            # sum1[p, idx] = sum_j E, sum2[p, idx] = sum_j E^2 (idx = 4b + c)
            # via 4x-mode tensor_scalar+accum; E^2 split DVE/GPSIMD.
            sum1 = sums.tile([128, 4 * BS], F32, tag="sum1")
            sum2 = sums.tile([128, 4 * BS], F32, tag="sum2")

            with (
                tc.tile_pool(name="ps_big", bufs=2, space="PSUM") as ps_big,
                tc.tile_pool(name="epool", bufs=4) as epool,
                tc.tile_pool(name="jpool", bufs=2) as jpool,
            ):
                for b in range(BS):
                    psP = ps_big.tile([128, 4, REP], F32, tag="psP")
                    off = REP * b
                    for c in range(4):
                        if b == 0:
                            lhs = s_bf2[0:1, 128 * c:128 * (c + 1)]
                            rhs = t_bf2[0:1, :]
                        else:
                            lhs = s_flat[0:1, off + 128 * c:off + 128 * (c + 1)]
                            rhs = t_flat[0:1, off:off + REP]
                        nc.tensor.matmul(
                            psP[:, c, :], lhs, rhs, start=True, stop=True,
                        )
                    E = epool.tile([128, 4, REP], BF16, tag="E")
                    nc.scalar.activation(out=E, in_=psP, func=AF.Exp)
                    if b == BS - 1:
                        nc.scalar.activation(
                            out=dummy, in_=E[0:1, 0, 0:1], func=AF.Ln)
                    E2a = epool.tile([128, 2, REP], BF16, tag="E2a")
                    E2b = epool.tile([128, 2, REP], BF16, tag="E2b")
                    junk = jpool.tile([128, REP], BF16, tag="junk")
                    nc.vector.tensor_tensor(
                        out=E2a, in0=E[:, 0:2, :], in1=E[:, 0:2, :], op=OP.mult)
                    nc.gpsimd.tensor_tensor(
                        out=E2b, in0=E[:, 2:4, :], in1=E[:, 2:4, :], op=OP.mult)
                    for c in range(4):
                        idx = 4 * b + c
                        nc.vector.tensor_scalar(
                            out=junk, in0=E[:, c, :], scalar1=1.0, scalar2=None,
                            op0=OP.mult, op1=OP.add,
                            accum_out=sum1[:, idx:idx + 1])
                    for c in range(4):
                        idx = 4 * b + c
                        e2src = E2a[:, c, :] if c < 2 else E2b[:, c - 2, :]
                        nc.vector.tensor_scalar(
                            out=junk, in0=e2src, scalar1=1.0, scalar2=None,
                            op0=OP.mult, op1=OP.add,
                            accum_out=sum2[:, idx:idx + 1])


            # ---- finalize: v = sum_cols( 2 ln(sum1) - ln(sum2) ) ----
            lg1 = sums.tile([128, 4 * BS], F32, tag="lg1")
            lg2 = sums.tile([128, 4 * BS], F32, tag="lg2")
            nc.scalar.activation(out=lg1, in_=sum1, func=AF.Ln)
            nc.scalar.activation(out=lg2, in_=sum2, func=AF.Ln)
            cg = sums.tile([128, 4 * BS], F32, tag="cg")
            v_sb = sums.tile([128, 1], F32, tag="v")
            nc.vector.scalar_tensor_tensor(
                out=cg, in0=lg1, scalar=2.0, in1=lg2,
                op0=OP.mult, op1=OP.subtract, accum_out=v_sb,
            )
            nc.sync.dma_start(out=v_out, in_=v_sb)

    nc.compile()
    return nc


_NC = None


def _get_nc():
    global _NC
    if _NC is None:
        _NC = build_program()
    return _NC


def make_in_maps(state, next_state, W1, b1, W2, b2, W3, b3):
    bf = ml_dtypes.bfloat16
    xT = np.asarray(state, np.float32).T
    yT = np.asarray(next_state, np.float32).T
    xyT = np.ascontiguousarray(np.concatenate([xT, yT], axis=1)).astype(bf)
    w1b = np.asarray(W1, np.float32).astype(bf)
    f8 = np.dtype(mybir.dt.np(F8))
    w2b = (np.asarray(W2, np.float32) * WSCALE).astype(f8)
    w3b = (np.asarray(W3, np.float32) * WSCALE).astype(f8)
    # b2/b3 ride the pre-descale PSUM, so pre-scale them to compensate
    bcat = np.concatenate([
        np.asarray(b1, np.float32),
        np.asarray(b2, np.float32) * WSCALE,
        np.asarray(b3, np.float32) * WSCALE,
    ])
    in_maps = []
    for c in range(NCORES):
        sl = slice(c * BS, (c + 1) * BS)
        in_maps.append({
            "xyT": xyT,
            "xycT": np.ascontiguousarray(
                np.concatenate([xT[:, sl], yT[:, sl]], axis=1)).astype(bf),
            "w1": w1b, "w2": w2b, "w3": w3b, "bcat": bcat,
        })
    return in_maps


def kernel(state, next_state, W1, b1, W2, b2, W3, b3, _trace=False, _tmpdir=None):
    nc = _get_nc()
    in_maps = make_in_maps(state, next_state, W1, b1, W2, b2, W3, b3)
    res = run_bass_kernel_spmd(
        nc, in_maps, list(range(NCORES)), trace=_trace, tmpdir=_tmpdir
    )
    total = np.float64(0.0)
    for c in range(NCORES):
        total += np.asarray(res.results[c]["v"], np.float64).sum()
    out = np.array(np.float32(total))
    if _trace:
        out_res = (out, res)
        return out_res
    return out
